# revision 20
# baseline (speedup 1.0000x reference)
"""GroupedQueryAttention Trainium2 kernel (8 NeuronCores), v2.

Sharding: core i handles (batch b = i//4, KV group g = i%4): its 4 query
heads + 1 KV group, full sequence. Each core computes a partial output
(attn_heads @ Wo rows for its heads) in bf16; host sums the 4 partials
per batch in fp32.

v2 layout strategy (per core), all PE inputs bf16 (PSUM accum fp32):
  - projections: W-stationary matmuls over 16 e-chunks, moving x bf16.
  - RoPE: host permutes W rows to half-split layout; swap-half via a
    permutation matmul on PE; raw+swapped copied psum->sbuf bf16 on ACT;
    cos/sin multiplies+add on DVE at 4x bf16 rate.
  - attention: per (bi, h): j-pairs share a [128,1024] psum tile (two
    512-wide score matmuls), one batched exp on ACT per non-diag pair,
    denominators via ones-matmul and PV accumulated into a combined
    [128,1024] psum tile (pso | psd); normalization on DVE.
  - out projection: per 128-row tq tile, 4x 512-col accumulation chains
    into two [128,1024] psum tiles, drained by DVE to bf16, one DMA per
    tq row-block. Interleaved per bi-block right after its attention.
PSUM: exactly 2 pools x [128,1024] f32 x 2 bufs = 8 banks.
"""

import numpy as np
import ml_dtypes
from contextlib import ExitStack

import concourse.bass as bass
import concourse.bacc as bacc
import concourse.tile as tile
import concourse.mybir as mybir
from concourse.bass_utils import run_bass_kernel_spmd

# problem shape (hardcoded per contract)
B, T, E = 2, 2048, 2048
NH, NG, HD = 16, 4, 128
HPG = NH // NG          # 4 heads per group = per core
NE = E // 128           # 16 contraction chunks
TB = 512                # tq / t block
NTB = T // TB           # 4
F32 = mybir.dt.float32
BF16 = mybir.dt.bfloat16
EXP = mybir.ActivationFunctionType.Exp
NPBF16 = ml_dtypes.bfloat16

N_CORES = 8


def build_body(tc, out_ap, ins):
    """ins: dict name -> dram AP. out_ap: [T, E] dram AP (bf16)."""
    nc = tc.nc
    ctx = ExitStack()
    with ctx:
        ctx.enter_context(nc.allow_low_precision(
            reason="bf16 matmul inputs / outputs are intended"))

        # ---- constant / persistent SBUF ----
        const = ctx.enter_context(tc.tile_pool(name="const", bufs=1))
        cs2 = const.tile([128, T], BF16, tag="cs2", name="cs2")
        snpm = const.tile([128, T], BF16, tag="snpm", name="snpm")
        tri = const.tile([128, 128], BF16, tag="tri", name="tri")
        swp = const.tile([128, 128], BF16, tag="swp", name="swp")
        iden = const.tile([128, 128], F32, tag="iden", name="iden")
        ones = const.tile([128, 128], BF16, tag="ones", name="ones")

        persist = ctx.enter_context(tc.tile_pool(name="persist", bufs=1))
        # packed x: [128, tb*8192 + e*512 + c]
        xb = persist.tile([128, NTB * NE * TB], BF16, tag="xb", name="xb")
        big = persist.tile([128, 6 * T], BF16, tag="big", name="big")
        qrot = [big[:, h * T:(h + 1) * T] for h in range(HPG)]
        krot = big[:, 4 * T:5 * T]
        vsd = big[:, 5 * T:6 * T]
        aout = qrot  # attn output overwrites qrot block-by-block

        # ---- weights (packed by host into sbuf layout) ----
        wpool = ctx.enter_context(tc.tile_pool(name="weights", bufs=1))
        wq_t = wpool.tile([128, NE * 512], BF16, tag="wq", name="wq")
        wk_t = wpool.tile([128, NE * 128], BF16, tag="wk", name="wk")
        wv_t = wpool.tile([128, NE * 128], BF16, tag="wv", name="wv")
        wo_t = wpool.tile([128, NE * 512], BF16, tag="wo", name="wo")

        # ---- psum pools: 2 pools x [128,1024] x 2 bufs = 8 banks ----
        PB = ctx.enter_context(tc.tile_pool(name="pb", bufs=2, space="PSUM"))
        PD = ctx.enter_context(tc.tile_pool(name="pd", bufs=2, space="PSUM"))

        # ---- sbuf working pools ----
        rawp = ctx.enter_context(tc.tile_pool(name="rawp", bufs=3))
        ptp = ctx.enter_context(tc.tile_pool(name="ptp", bufs=3))
        rdp = ctx.enter_context(tc.tile_pool(name="rdp", bufs=2))
        osp = ctx.enter_context(tc.tile_pool(name="osp", bufs=2))

        # ---------------- DMA preload ----------------
        # Each issuing engine owns a ~140GB/s DMA queue; split the load three
        # ways and order by first use so the e-interleaved projection streams
        # against DMA arrival.
        # gpsimd queue: x, 4 progressive chunks per tb (e-range of 4 each).
        for tb in range(NTB):
            nch = 8 if tb == 0 else 4
            for c in range(nch):
                w = NE * TB // nch
                base = tb * NE * TB + c * w
                nc.gpsimd.dma_start(xb[:, base:base + w],
                                    ins["xb"][:, base:base + w])
        # sync queue: wk/wv halves, transpose/mask consts, wo.
        nc.sync.dma_start(wk_t[:, 0:NE * 64], ins["wk"][:, 0:NE * 64])
        nc.sync.dma_start(wk_t[:, NE * 64:NE * 128], ins["wk"][:, NE * 64:NE * 128])
        nc.sync.dma_start(wv_t[:, 0:NE * 64], ins["wv"][:, 0:NE * 64])
        nc.sync.dma_start(wv_t[:, NE * 64:NE * 128], ins["wv"][:, NE * 64:NE * 128])
        nc.sync.dma_start(iden[:], ins["iden"][:])
        nc.sync.dma_start(tri[:], ins["tri"][:])
        nc.sync.dma_start(ones[:], ins["ones"][:])
        nc.sync.dma_start(wo_t[:], ins["wo"][:])
        # scalar queue: swp, wq quarters (progressive by e), rope tables.
        nc.scalar.dma_start(swp[:], ins["swp"][:])
        for c in range(2):
            nc.scalar.dma_start(wq_t[:, c * 2 * 512:(c + 1) * 2 * 512],
                                ins["wq"][:, c * 2 * 512:(c + 1) * 2 * 512])
        for c in range(1, 4):
            nc.scalar.dma_start(wq_t[:, c * 4 * 512:(c + 1) * 4 * 512],
                                ins["wq"][:, c * 4 * 512:(c + 1) * 4 * 512])
        nc.scalar.dma_start(cs2[:], ins["cs2"][:])
        nc.scalar.dma_start(snpm[:], ins["snpm"][:])

        def xc(tb, e):
            base = tb * NE * TB + e * TB
            return xb[:, base:base + TB]

        def rope_copy(ps, tag):
            """Drain psum projection [128, TB] to a bf16 sbuf tile on ACT."""
            raw = rawp.tile([128, TB], BF16, tag="raw", bufs=6, name=f"raw{tag}")
            nc.scalar.copy(raw[:], ps)
            return raw

        def rope_finish(dst_ap, raw, psw_half, cols, tag):
            """dst = raw*cos + swap(raw)*sgn_sin; swap via PE perm matmul."""
            nc.tensor.matmul(psw_half, swp[:], raw[:], start=True, stop=True)
            sw = rawp.tile([128, TB], BF16, tag="sw", name=f"sw{tag}")
            nc.scalar.copy(sw[:], psw_half)
            tmp1 = rawp.tile([128, TB], BF16, tag="tmp", bufs=2, name=f"t1{tag}")
            tmp2 = rawp.tile([128, TB], BF16, tag="tmp", bufs=2, name=f"t2{tag}")
            nc.vector.tensor_mul(tmp1[:], raw[:], cs2[:, cols])
            nc.vector.tensor_mul(tmp2[:], sw[:], snpm[:, cols])
            nc.vector.tensor_add(dst_ap, tmp1[:], tmp2[:])

        def jmeta(bi):
            jorder = list(range(4 * bi, 4 * bi + 4)) + list(range(4 * bi))
            return [(jorder[2 * p], jorder[2 * p + 1])
                    for p in range(len(jorder) // 2)]

        def off(bi, j):
            return 128 * (j - 4 * bi) if j >= 4 * bi else 0

        def scores_exp(bi, h, jp):
            j0, j1 = jmeta(bi)[jp]
            o0, o1 = off(bi, j0), off(bi, j1)
            pb = PB.tile([128, 1024], F32, tag="b", name="pb")
            nc.tensor.matmul(
                pb[:, 0:TB - o0],
                krot[:, j0 * 128:(j0 + 1) * 128],
                qrot[h][:, bi * TB + o0:(bi + 1) * TB],
                start=True, stop=True)
            nc.tensor.matmul(
                pb[:, TB:2 * TB - o1],
                krot[:, j1 * 128:(j1 + 1) * 128],
                qrot[h][:, bi * TB + o1:(bi + 1) * TB],
                start=True, stop=True)
            pt = ptp.tile([128, 1024], BF16, tag="pt", name="pt")
            if o0 == 0 and o1 == 0:
                nc.scalar.activation(pt[:], pb[:], EXP)
            else:
                nc.scalar.activation(pt[:, o0:TB], pb[:, 0:TB - o0], EXP)
                nc.scalar.activation(pt[:, TB + o1:2 * TB],
                                     pb[:, TB:2 * TB - o1], EXP)
            for half, j, o in ((0, j0, o0), (1, j1, o1)):
                if j >= 4 * bi:  # diagonal tile: causal mask
                    c0 = half * TB
                    nc.vector.tensor_mul(pt[:, c0 + o:c0 + o + 128],
                                         pt[:, c0 + o:c0 + o + 128], tri[:])
            return pt

        def denom_pv(bi, h, jp, pa, pt):
            j0, j1 = jmeta(bi)[jp]
            o0, o1 = off(bi, j0), off(bi, j1)
            last = (jp == 2 * bi + 1)
            nc.tensor.matmul(pa[:, TB + o0:2 * TB], ones[:],
                             pt[:, o0:TB], start=(jp == 0), stop=False)
            nc.tensor.matmul(pa[:, TB + o1:2 * TB], ones[:],
                             pt[:, TB + o1:2 * TB], start=False, stop=last)
            nc.tensor.matmul(pa[:, o0:TB], vsd[:, j0 * 128:(j0 + 1) * 128],
                             pt[:, o0:TB], start=(jp == 0), stop=False)
            nc.tensor.matmul(pa[:, o1:TB], vsd[:, j1 * 128:(j1 + 1) * 128],
                             pt[:, TB + o1:2 * TB], start=False, stop=last)

        from collections import deque
        ptq = deque()
        AHEAD = 2
        all_units = {bi: [(h, jp) for h in range(HPG) for jp in range(2 * bi + 2)]
                     for bi in range(NTB)}

        def prologue():
            for k in range(AHEAD):
                h, jp = all_units[0][k]
                ptq.append(scores_exp(0, h, jp))

        # ================= projection phase =================
        # First pass interleaves chains k, v, q0, q1 by e-chunk so the PE
        # consumes x/wq in DMA arrival order (no big re-scan stalls on tb0);
        # q2/q3 run as a second pass (into a PB tile) over the by-then
        # resident x block while ACT drains the first-pass psums. RoPE is
        # split into ACT copy-out (right after each chain) and the
        # swap-matmul/DVE math (emitted after the q2/q3 chains) so the PE
        # never waits on ACT.
        for tb in range(NTB):
            cols = slice(tb * TB, (tb + 1) * TB)
            pkv = PD.tile([128, 1024], F32, tag="d", name="pkv")
            pq01 = PD.tile([128, 1024], F32, tag="d", name="pq01")
            for e in range(NE):
                st, sp = (e == 0), (e == NE - 1)
                nc.tensor.matmul(pkv[:, 0:TB], wk_t[:, e * 128:(e + 1) * 128],
                                 xc(tb, e), start=st, stop=sp)
                nc.tensor.matmul(pkv[:, TB:2 * TB], wv_t[:, e * 128:(e + 1) * 128],
                                 xc(tb, e), start=st, stop=sp)
                for k in range(2):
                    nc.tensor.matmul(
                        pq01[:, k * TB:(k + 1) * TB],
                        wq_t[:, e * 512 + k * 128: e * 512 + (k + 1) * 128],
                        xc(tb, e), start=st, stop=sp)
            raw_k = rope_copy(pkv[:, 0:TB], "k")
            vtmp = rawp.tile([128, TB], F32, tag="vtmp", bufs=2, name="vtmp")
            nc.scalar.copy(vtmp[:], pkv[:, TB:2 * TB])
            raw_q0 = rope_copy(pq01[:, 0:TB], "q0")
            raw_q1 = rope_copy(pq01[:, TB:2 * TB], "q1")

            pq23 = PB.tile([128, 1024], F32, tag="b", name="pq23")
            for k in range(2):
                h = 2 + k
                for e in range(NE):
                    nc.tensor.matmul(
                        pq23[:, k * TB:(k + 1) * TB],
                        wq_t[:, e * 512 + h * 128: e * 512 + (h + 1) * 128],
                        xc(tb, e), start=(e == 0), stop=(e == NE - 1))
            raw_q2 = rope_copy(pq23[:, 0:TB], "q2")
            raw_q3 = rope_copy(pq23[:, TB:2 * TB], "q3")

            psw1 = PB.tile([128, 1024], F32, tag="b", name="psw1")
            psw3 = PD.tile([128, 1024], F32, tag="d", name="psw3")
            # transpose v tiles first so the vsd copy leads the DVE queue
            for jj in range(4):
                nc.tensor.transpose(psw3[:, TB + jj * 128:TB + (jj + 1) * 128],
                                    vtmp[:, jj * 128:(jj + 1) * 128], iden[:])
            nc.vector.tensor_copy(vsd[:, cols], psw3[:, TB:2 * TB])
            psw2 = PD.tile([128, 1024], F32, tag="d", name="psw2")
            rope_finish(qrot[3][:, cols], raw_q3, psw3[:, 0:TB], cols, "q3")
            if tb == NTB - 1:
                rope_finish(krot[:, cols], raw_k, psw1[:, 0:TB], cols, "k")
                rope_finish(qrot[0][:, cols], raw_q0, psw1[:, TB:2 * TB], cols, "q0")
                prologue()
                rope_finish(qrot[1][:, cols], raw_q1, psw2[:, 0:TB], cols, "q1")
                rope_finish(qrot[2][:, cols], raw_q2, psw2[:, TB:2 * TB], cols, "q2")
            else:
                rope_finish(qrot[1][:, cols], raw_q1, psw2[:, 0:TB], cols, "q1")
                rope_finish(qrot[2][:, cols], raw_q2, psw2[:, TB:2 * TB], cols, "q2")
                rope_finish(krot[:, cols], raw_k, psw1[:, 0:TB], cols, "k")
                rope_finish(qrot[0][:, cols], raw_q0, psw1[:, TB:2 * TB], cols, "q0")


        # ================= attention + out-projection =================
        # Flat software pipeline: the scores+exp producer runs AHEAD units
        # in front of the denominator/PV consumer, across head and bi-block
        # boundaries (the next block's first two units are emitted before
        # this block's out-projection so ACT exps while PE projects).
        for bi in range(NTB):
            qcols = slice(bi * TB, (bi + 1) * TB)
            units = all_units[bi]
            pa = None
            for i, (h, jp) in enumerate(units):
                if i + AHEAD < len(units):
                    h2, jp2 = units[i + AHEAD]
                    ptq.append(scores_exp(bi, h2, jp2))
                elif bi + 1 < NTB:
                    h2, jp2 = all_units[bi + 1][i + AHEAD - len(units)]
                    ptq.append(scores_exp(bi + 1, h2, jp2))
                if jp == 0:
                    pa = PD.tile([128, 1024], F32, tag="d", name="pa")
                denom_pv(bi, h, jp, pa, ptq.popleft())
                if jp == 2 * bi + 1:
                    rden = rdp.tile([128, TB], F32, tag="rden", name="rden")
                    nc.vector.reciprocal_approx_fast(rden[:], pa[:, TB:2 * TB])
                    nc.vector.tensor_mul(aout[h][:, qcols], pa[:, 0:TB], rden[:])

            # out-projection for this bi block
            for tq in range(4):
                trows = slice(bi * TB + tq * 128, bi * TB + (tq + 1) * 128)
                po = [PD.tile([128, 1024], F32, tag="d", name="po") for _ in range(2)]
                for eo in range(4):
                    tgt = po[eo // 2][:, (eo % 2) * TB:(eo % 2 + 1) * TB]
                    for hh in range(HPG):
                        nc.tensor.matmul(
                            tgt, aout[hh][:, trows],
                            wo_t[:, (hh * 4 + eo) * 512:(hh * 4 + eo + 1) * 512],
                            start=(hh == 0), stop=(hh == HPG - 1))
                osb = osp.tile([128, 2048], BF16, tag="osb", name="osb")
                nc.scalar.copy(osb[:, 0:1024], po[0][:])
                nc.vector.tensor_copy(osb[:, 1024:2048], po[1][:])
                nc.sync.dma_start(out_ap[trows, :], osb[:])


# ---------------- host side ----------------

_PERM = np.concatenate([np.arange(0, HD, 2), np.arange(1, HD, 2)])  # half-split


def _pack_w(w):
    """[E, C] -> [128, NE*C] sbuf layout (col block = e-chunk)."""
    c = w.shape[1]
    return np.ascontiguousarray(
        w.reshape(NE, 128, c).transpose(1, 0, 2).reshape(128, NE * c)
    ).astype(NPBF16)


def host_prep(inputs):
    """Full inputs -> list of 8 per-core input dicts (core i = (b=i//4, g=i%4))."""
    x = np.asarray(inputs["x"], dtype=np.float32)
    Wq = np.asarray(inputs["Wq"], dtype=np.float32)
    Wk = np.asarray(inputs["Wk"], dtype=np.float32)
    Wv = np.asarray(inputs["Wv"], dtype=np.float32)
    Wo = np.asarray(inputs["Wo"], dtype=np.float32)

    inv = (10000.0 ** (-np.arange(0, HD, 2, dtype=np.float32) / HD)).astype(np.float32)
    tpos = np.arange(T, dtype=np.float32)
    fr = np.outer(tpos, inv)                       # [T, 64]
    cosT = np.cos(fr).T.astype(np.float32)         # [64, T]
    sinT = np.sin(fr).T.astype(np.float32)
    cs2 = np.concatenate([cosT, cosT], axis=0).astype(NPBF16)     # [128, T]
    snpm = np.concatenate([-sinT, sinT], axis=0).astype(NPBF16)   # [128, T]

    tri = (np.arange(128)[None, :] >= np.arange(128)[:, None]).astype(NPBF16)
    swp = np.zeros((128, 128), dtype=np.float32)
    swp[(np.arange(128) + 64) % 128, np.arange(128)] = 1.0
    swp = swp.astype(NPBF16)
    iden = np.eye(128, dtype=np.float32)
    ones = np.ones((128, 128), dtype=np.float32).astype(NPBF16)

    scale = np.float32(1.0 / np.sqrt(HD))
    # xb[b]: [128, tb*8192 + e*512 + c] = x[b][tb*512+c, e*128+p]
    xbs = []
    for b in range(B):
        xT = x[b].T                                  # [E, T]
        v = xT.reshape(NE, 128, NTB, TB).transpose(1, 2, 0, 3)
        xbs.append(np.ascontiguousarray(v.reshape(128, NTB * NE * TB)).astype(NPBF16))

    in_maps = []
    for i in range(N_CORES):
        b, g = i // 4, i % 4
        rows = []
        for h in range(HPG):
            base = (g * HPG + h) * HD
            rows.append(Wq[base + _PERM, :])
        wq_c = (np.concatenate(rows, axis=0) * scale).T  # [E, 512]
        wk_c = Wk[g * HD + _PERM, :].T                   # [E, 128]
        wv_c = Wv[g * HD:(g + 1) * HD, :].T              # [E, 128]
        # wo blocks (hh, eo): [128, (hh*4+eo)*512 + c] = WoT[hh*128+p, eo*512+c]
        wo_c = Wo[:, g * 512:(g + 1) * 512].T            # [512, E]
        wo_p = np.ascontiguousarray(
            wo_c.reshape(HPG, 128, 4, 512).transpose(1, 0, 2, 3).reshape(128, NE * 512)
        ).astype(NPBF16)
        in_maps.append({
            "xb": xbs[b],
            "wq": _pack_w(wq_c),
            "wk": _pack_w(wk_c),
            "wv": _pack_w(wv_c),
            "wo": wo_p,
            "cs2": cs2, "snpm": snpm, "tri": tri, "swp": swp, "iden": iden,
            "ones": ones,
        })
    return in_maps


_NC = None


def build_nc():
    global _NC
    if _NC is not None:
        return _NC
    nc = bacc.Bacc("TRN2", target_bir_lowering=False, debug=False,
                   num_devices=N_CORES)
    ins = {
        "xb": nc.dram_tensor("xb", [128, NTB * NE * TB], BF16, kind="ExternalInput").ap(),
        "wq": nc.dram_tensor("wq", [128, NE * 512], BF16, kind="ExternalInput").ap(),
        "wk": nc.dram_tensor("wk", [128, NE * 128], BF16, kind="ExternalInput").ap(),
        "wv": nc.dram_tensor("wv", [128, NE * 128], BF16, kind="ExternalInput").ap(),
        "wo": nc.dram_tensor("wo", [128, NE * 512], BF16, kind="ExternalInput").ap(),
        "cs2": nc.dram_tensor("cs2", [128, T], BF16, kind="ExternalInput").ap(),
        "snpm": nc.dram_tensor("snpm", [128, T], BF16, kind="ExternalInput").ap(),
        "tri": nc.dram_tensor("tri", [128, 128], BF16, kind="ExternalInput").ap(),
        "swp": nc.dram_tensor("swp", [128, 128], BF16, kind="ExternalInput").ap(),
        "iden": nc.dram_tensor("iden", [128, 128], F32, kind="ExternalInput").ap(),
        "ones": nc.dram_tensor("ones", [128, 128], BF16, kind="ExternalInput").ap(),
    }
    out = nc.dram_tensor("out", [T, E], BF16, kind="ExternalOutput").ap()
    with tile.TileContext(nc) as tc:
        build_body(tc, out, ins)
    nc.compile()
    _NC = nc
    return nc


def gather(results):
    """results: list of 8 dicts with 'out' [T, E] bf16 partials -> [B, T, E] f32."""
    out = np.zeros((B, T, E), dtype=np.float32)
    for i in range(N_CORES):
        out[i // 4] += np.asarray(results[i]["out"], dtype=np.float32)
    return out


def kernel(**inputs):
    nc = build_nc()
    in_maps = host_prep(inputs)
    res = run_bass_kernel_spmd(nc, in_maps, core_ids=list(range(N_CORES)))
    return gather(res.results)


if __name__ == "__main__":
    rng = np.random.default_rng(0)
    ins = {
        "x": rng.standard_normal((B, T, E), dtype=np.float32),
        "Wq": rng.standard_normal((E, E), dtype=np.float32) * 0.02,
        "Wk": rng.standard_normal((NG * HD, E), dtype=np.float32) * 0.02,
        "Wv": rng.standard_normal((NG * HD, E), dtype=np.float32) * 0.02,
        "Wo": rng.standard_normal((E, E), dtype=np.float32) * 0.02,
    }
    out = kernel(**ins)
    print(out.shape, out.dtype, np.abs(out).mean())


# revision 22
# speedup vs baseline: 1.0032x; 1.0032x over previous
"""GroupedQueryAttention Trainium2 kernel (8 NeuronCores), v2.

Sharding: core i handles (batch b = i//4, KV group g = i%4): its 4 query
heads + 1 KV group, full sequence. Each core computes a partial output
(attn_heads @ Wo rows for its heads) in bf16; host sums the 4 partials
per batch in fp32.

v2 layout strategy (per core), all PE inputs bf16 (PSUM accum fp32):
  - projections: W-stationary matmuls over 16 e-chunks, moving x bf16.
  - RoPE: host permutes W rows to half-split layout; swap-half via a
    permutation matmul on PE; raw+swapped copied psum->sbuf bf16 on ACT;
    cos/sin multiplies+add on DVE at 4x bf16 rate.
  - attention: per (bi, h): j-pairs share a [128,1024] psum tile (two
    512-wide score matmuls), one batched exp on ACT per non-diag pair,
    denominators via ones-matmul and PV accumulated into a combined
    [128,1024] psum tile (pso | psd); normalization on DVE.
  - out projection: per 128-row tq tile, 4x 512-col accumulation chains
    into two [128,1024] psum tiles, drained by DVE to bf16, one DMA per
    tq row-block. Interleaved per bi-block right after its attention.
PSUM: exactly 2 pools x [128,1024] f32 x 2 bufs = 8 banks.
"""

import numpy as np
import ml_dtypes
from contextlib import ExitStack

import concourse.bass as bass
import concourse.bacc as bacc
import concourse.tile as tile
import concourse.mybir as mybir
from concourse.bass_utils import run_bass_kernel_spmd

# problem shape (hardcoded per contract)
B, T, E = 2, 2048, 2048
NH, NG, HD = 16, 4, 128
HPG = NH // NG          # 4 heads per group = per core
NE = E // 128           # 16 contraction chunks
TB = 512                # tq / t block
NTB = T // TB           # 4
F32 = mybir.dt.float32
BF16 = mybir.dt.bfloat16
EXP = mybir.ActivationFunctionType.Exp
NPBF16 = ml_dtypes.bfloat16

N_CORES = 8


def build_body(tc, out_ap, ins):
    """ins: dict name -> dram AP. out_ap: [T, E] dram AP (bf16)."""
    nc = tc.nc
    ctx = ExitStack()
    with ctx:
        ctx.enter_context(nc.allow_low_precision(
            reason="bf16 matmul inputs / outputs are intended"))

        # ---- constant / persistent SBUF ----
        const = ctx.enter_context(tc.tile_pool(name="const", bufs=1))
        cs2 = const.tile([128, T], BF16, tag="cs2", name="cs2")
        snpm = const.tile([128, T], BF16, tag="snpm", name="snpm")
        tri = const.tile([128, 128], BF16, tag="tri", name="tri")
        iden = const.tile([128, 128], F32, tag="iden", name="iden")
        ones = const.tile([128, 128], BF16, tag="ones", name="ones")

        persist = ctx.enter_context(tc.tile_pool(name="persist", bufs=1))
        # packed x: [128, tb*8192 + e*512 + c]
        xb = persist.tile([128, NTB * NE * TB], BF16, tag="xb", name="xb")
        big = persist.tile([128, 6 * T], BF16, tag="big", name="big")
        qrot = [big[:, h * T:(h + 1) * T] for h in range(HPG)]
        krot = big[:, 4 * T:5 * T]
        vsd = big[:, 5 * T:6 * T]
        aout = qrot  # attn output overwrites qrot block-by-block

        # ---- weights (packed by host into sbuf layout) ----
        wpool = ctx.enter_context(tc.tile_pool(name="weights", bufs=1))
        wq_t = wpool.tile([128, NE * 512], BF16, tag="wq", name="wq")
        wk_t = wpool.tile([128, NE * 128], BF16, tag="wk", name="wk")
        wv_t = wpool.tile([128, NE * 128], BF16, tag="wv", name="wv")
        wo_t = wpool.tile([128, NE * 512], BF16, tag="wo", name="wo")

        # ---- psum pools: 2 pools x [128,1024] x 2 bufs = 8 banks ----
        PB = ctx.enter_context(tc.tile_pool(name="pb", bufs=2, space="PSUM"))
        PD = ctx.enter_context(tc.tile_pool(name="pd", bufs=2, space="PSUM"))

        # ---- sbuf working pools ----
        rawp = ctx.enter_context(tc.tile_pool(name="rawp", bufs=3))
        ptp = ctx.enter_context(tc.tile_pool(name="ptp", bufs=3))
        rdp = ctx.enter_context(tc.tile_pool(name="rdp", bufs=2))
        osp = ctx.enter_context(tc.tile_pool(name="osp", bufs=2))

        # ---------------- DMA preload ----------------
        # Each issuing engine owns a ~140GB/s DMA queue; split the load three
        # ways and order by first use so the e-interleaved projection streams
        # against DMA arrival.
        # gpsimd queue: x, 4 progressive chunks per tb (e-range of 4 each).
        for tb in range(NTB):
            nch = 8 if tb == 0 else 4
            for c in range(nch):
                w = NE * TB // nch
                base = tb * NE * TB + c * w
                nc.gpsimd.dma_start(xb[:, base:base + w],
                                    ins["xb"][:, base:base + w])
        # sync queue: wk/wv halves, transpose/mask consts, wo.
        nc.sync.dma_start(wk_t[:, 0:NE * 64], ins["wk"][:, 0:NE * 64])
        nc.sync.dma_start(wk_t[:, NE * 64:NE * 128], ins["wk"][:, NE * 64:NE * 128])
        nc.sync.dma_start(wv_t[:, 0:NE * 64], ins["wv"][:, 0:NE * 64])
        nc.sync.dma_start(wv_t[:, NE * 64:NE * 128], ins["wv"][:, NE * 64:NE * 128])
        nc.sync.dma_start(iden[:], ins["iden"][:])
        nc.sync.dma_start(tri[:], ins["tri"][:])
        nc.sync.dma_start(ones[:], ins["ones"][:])
        nc.sync.dma_start(wo_t[:], ins["wo"][:])
        # scalar queue: wq quarters (progressive by e), rope tables.
        for c in range(2):
            nc.scalar.dma_start(wq_t[:, c * 2 * 512:(c + 1) * 2 * 512],
                                ins["wq"][:, c * 2 * 512:(c + 1) * 2 * 512])
        for c in range(1, 4):
            nc.scalar.dma_start(wq_t[:, c * 4 * 512:(c + 1) * 4 * 512],
                                ins["wq"][:, c * 4 * 512:(c + 1) * 4 * 512])
        nc.scalar.dma_start(cs2[:], ins["cs2"][:])
        nc.scalar.dma_start(snpm[:], ins["snpm"][:])

        def xc(tb, e):
            base = tb * NE * TB + e * TB
            return xb[:, base:base + TB]

        def rope_copy(ps, tag):
            """Drain psum projection [128, TB] to bf16 sbuf on ACT, then swap
            halves across partitions via SBUF->SBUF DMAs on the sync queue
            (DMA is address-based, so it can cross partitions; the swapped
            copy is consumed later by the deferred rope_finish on DVE)."""
            raw = rawp.tile([128, TB], BF16, tag="raw", bufs=6, name=f"raw{tag}")
            nc.scalar.copy(raw[:], ps)
            sw = rawp.tile([128, TB], BF16, tag="sw", bufs=6, name=f"sw{tag}")
            nc.sync.dma_start(sw[0:64, :], raw[64:128, :])
            nc.sync.dma_start(sw[64:128, :], raw[0:64, :])
            return raw, sw

        def rope_finish(dst_ap, rs, cols, tag):
            """dst = raw*cos + swap(raw)*sgn_sin (all DVE, bf16 4x rate)."""
            raw, sw = rs
            tmp1 = rawp.tile([128, TB], BF16, tag="tmp", bufs=2, name=f"t1{tag}")
            tmp2 = rawp.tile([128, TB], BF16, tag="tmp", bufs=2, name=f"t2{tag}")
            nc.vector.tensor_mul(tmp1[:], raw[:], cs2[:, cols])
            nc.vector.tensor_mul(tmp2[:], sw[:], snpm[:, cols])
            nc.vector.tensor_add(dst_ap, tmp1[:], tmp2[:])

        def jmeta(bi):
            jorder = list(range(4 * bi, 4 * bi + 4)) + list(range(4 * bi))
            return [(jorder[2 * p], jorder[2 * p + 1])
                    for p in range(len(jorder) // 2)]

        def off(bi, j):
            return 128 * (j - 4 * bi) if j >= 4 * bi else 0

        def scores_exp(bi, h, jp):
            j0, j1 = jmeta(bi)[jp]
            o0, o1 = off(bi, j0), off(bi, j1)
            pb = PB.tile([128, 1024], F32, tag="b", name="pb")
            nc.tensor.matmul(
                pb[:, 0:TB - o0],
                krot[:, j0 * 128:(j0 + 1) * 128],
                qrot[h][:, bi * TB + o0:(bi + 1) * TB],
                start=True, stop=True)
            nc.tensor.matmul(
                pb[:, TB:2 * TB - o1],
                krot[:, j1 * 128:(j1 + 1) * 128],
                qrot[h][:, bi * TB + o1:(bi + 1) * TB],
                start=True, stop=True)
            pt = ptp.tile([128, 1024], BF16, tag="pt", name="pt")
            if o0 == 0 and o1 == 0:
                nc.scalar.activation(pt[:], pb[:], EXP)
            else:
                nc.scalar.activation(pt[:, o0:TB], pb[:, 0:TB - o0], EXP)
                nc.scalar.activation(pt[:, TB + o1:2 * TB],
                                     pb[:, TB:2 * TB - o1], EXP)
            for half, j, o in ((0, j0, o0), (1, j1, o1)):
                if j >= 4 * bi:  # diagonal tile: causal mask
                    c0 = half * TB
                    nc.vector.tensor_mul(pt[:, c0 + o:c0 + o + 128],
                                         pt[:, c0 + o:c0 + o + 128], tri[:])
            return pt

        def denom_pv(bi, h, jp, pa, pt):
            j0, j1 = jmeta(bi)[jp]
            o0, o1 = off(bi, j0), off(bi, j1)
            last = (jp == 2 * bi + 1)
            nc.tensor.matmul(pa[:, TB + o0:2 * TB], ones[:],
                             pt[:, o0:TB], start=(jp == 0), stop=False)
            nc.tensor.matmul(pa[:, TB + o1:2 * TB], ones[:],
                             pt[:, TB + o1:2 * TB], start=False, stop=last)
            nc.tensor.matmul(pa[:, o0:TB], vsd[:, j0 * 128:(j0 + 1) * 128],
                             pt[:, o0:TB], start=(jp == 0), stop=False)
            nc.tensor.matmul(pa[:, o1:TB], vsd[:, j1 * 128:(j1 + 1) * 128],
                             pt[:, TB + o1:2 * TB], start=False, stop=last)

        from collections import deque
        ptq = deque()
        AHEAD = 2
        all_units = {bi: [(h, jp) for h in range(HPG) for jp in range(2 * bi + 2)]
                     for bi in range(NTB)}

        def prologue():
            for k in range(AHEAD):
                h, jp = all_units[0][k]
                ptq.append(scores_exp(0, h, jp))

        # ================= projection phase =================
        # First pass interleaves chains k, v, q0, q1 by e-chunk so the PE
        # consumes x/wq in DMA arrival order (no big re-scan stalls on tb0);
        # q2/q3 run as a second pass (into a PB tile) over the by-then
        # resident x block while ACT drains the first-pass psums. RoPE is
        # split into ACT copy-out (right after each chain) and the
        # swap-matmul/DVE math (emitted after the q2/q3 chains) so the PE
        # never waits on ACT.
        for tb in range(NTB):
            cols = slice(tb * TB, (tb + 1) * TB)
            pkv = PD.tile([128, 1024], F32, tag="d", name="pkv")
            pq01 = PD.tile([128, 1024], F32, tag="d", name="pq01")
            for e in range(NE):
                st, sp = (e == 0), (e == NE - 1)
                nc.tensor.matmul(pkv[:, 0:TB], wk_t[:, e * 128:(e + 1) * 128],
                                 xc(tb, e), start=st, stop=sp)
                nc.tensor.matmul(pkv[:, TB:2 * TB], wv_t[:, e * 128:(e + 1) * 128],
                                 xc(tb, e), start=st, stop=sp)
                for k in range(2):
                    nc.tensor.matmul(
                        pq01[:, k * TB:(k + 1) * TB],
                        wq_t[:, e * 512 + k * 128: e * 512 + (k + 1) * 128],
                        xc(tb, e), start=st, stop=sp)
            raw_k = rope_copy(pkv[:, 0:TB], "k")
            vtmp = rawp.tile([128, TB], F32, tag="vtmp", bufs=2, name="vtmp")
            nc.scalar.copy(vtmp[:], pkv[:, TB:2 * TB])
            raw_q0 = rope_copy(pq01[:, 0:TB], "q0")
            raw_q1 = rope_copy(pq01[:, TB:2 * TB], "q1")

            pq23 = PB.tile([128, 1024], F32, tag="b", name="pq23")
            for k in range(2):
                h = 2 + k
                for e in range(NE):
                    nc.tensor.matmul(
                        pq23[:, k * TB:(k + 1) * TB],
                        wq_t[:, e * 512 + h * 128: e * 512 + (h + 1) * 128],
                        xc(tb, e), start=(e == 0), stop=(e == NE - 1))
            raw_q2 = rope_copy(pq23[:, 0:TB], "q2")
            raw_q3 = rope_copy(pq23[:, TB:2 * TB], "q3")

            ptv = PD.tile([128, 1024], F32, tag="d", name="ptv")
            # transpose v tiles first so the vsd copy leads the DVE queue
            for jj in range(4):
                nc.tensor.transpose(ptv[:, TB + jj * 128:TB + (jj + 1) * 128],
                                    vtmp[:, jj * 128:(jj + 1) * 128], iden[:])
            nc.vector.tensor_copy(vsd[:, cols], ptv[:, TB:2 * TB])
            if tb == NTB - 1:
                rope_finish(krot[:, cols], raw_k, cols, "k")
                rope_finish(qrot[0][:, cols], raw_q0, cols, "q0")
                prologue()
                rope_finish(qrot[1][:, cols], raw_q1, cols, "q1")
                rope_finish(qrot[2][:, cols], raw_q2, cols, "q2")
                rope_finish(qrot[3][:, cols], raw_q3, cols, "q3")
            else:
                rope_finish(qrot[1][:, cols], raw_q1, cols, "q1")
                rope_finish(qrot[2][:, cols], raw_q2, cols, "q2")
                rope_finish(krot[:, cols], raw_k, cols, "k")
                rope_finish(qrot[0][:, cols], raw_q0, cols, "q0")
                rope_finish(qrot[3][:, cols], raw_q3, cols, "q3")


        # ================= attention + out-projection =================
        # Flat software pipeline: the scores+exp producer runs AHEAD units
        # in front of the denominator/PV consumer, across head and bi-block
        # boundaries (the next block's first two units are emitted before
        # this block's out-projection so ACT exps while PE projects).
        for bi in range(NTB):
            qcols = slice(bi * TB, (bi + 1) * TB)
            units = all_units[bi]
            pa = None
            for i, (h, jp) in enumerate(units):
                if i + AHEAD < len(units):
                    h2, jp2 = units[i + AHEAD]
                    ptq.append(scores_exp(bi, h2, jp2))
                elif bi + 1 < NTB:
                    h2, jp2 = all_units[bi + 1][i + AHEAD - len(units)]
                    ptq.append(scores_exp(bi + 1, h2, jp2))
                if jp == 0:
                    pa = PD.tile([128, 1024], F32, tag="d", name="pa")
                denom_pv(bi, h, jp, pa, ptq.popleft())
                if jp == 2 * bi + 1:
                    rden = rdp.tile([128, TB], F32, tag="rden", name="rden")
                    nc.vector.reciprocal_approx_fast(rden[:], pa[:, TB:2 * TB])
                    nc.vector.tensor_mul(aout[h][:, qcols], pa[:, 0:TB], rden[:])

            # out-projection for this bi block
            for tq in range(4):
                trows = slice(bi * TB + tq * 128, bi * TB + (tq + 1) * 128)
                po = [PD.tile([128, 1024], F32, tag="d", name="po") for _ in range(2)]
                for eo in range(4):
                    tgt = po[eo // 2][:, (eo % 2) * TB:(eo % 2 + 1) * TB]
                    for hh in range(HPG):
                        nc.tensor.matmul(
                            tgt, aout[hh][:, trows],
                            wo_t[:, (hh * 4 + eo) * 512:(hh * 4 + eo + 1) * 512],
                            start=(hh == 0), stop=(hh == HPG - 1))
                osb = osp.tile([128, 2048], BF16, tag="osb", name="osb")
                nc.scalar.copy(osb[:, 0:1024], po[0][:])
                nc.vector.tensor_copy(osb[:, 1024:2048], po[1][:])
                nc.sync.dma_start(out_ap[trows, :], osb[:])


# ---------------- host side ----------------

_PERM = np.concatenate([np.arange(0, HD, 2), np.arange(1, HD, 2)])  # half-split


def _pack_w(w):
    """[E, C] -> [128, NE*C] sbuf layout (col block = e-chunk)."""
    c = w.shape[1]
    return np.ascontiguousarray(
        w.reshape(NE, 128, c).transpose(1, 0, 2).reshape(128, NE * c)
    ).astype(NPBF16)


def host_prep(inputs):
    """Full inputs -> list of 8 per-core input dicts (core i = (b=i//4, g=i%4))."""
    x = np.asarray(inputs["x"], dtype=np.float32)
    Wq = np.asarray(inputs["Wq"], dtype=np.float32)
    Wk = np.asarray(inputs["Wk"], dtype=np.float32)
    Wv = np.asarray(inputs["Wv"], dtype=np.float32)
    Wo = np.asarray(inputs["Wo"], dtype=np.float32)

    inv = (10000.0 ** (-np.arange(0, HD, 2, dtype=np.float32) / HD)).astype(np.float32)
    tpos = np.arange(T, dtype=np.float32)
    fr = np.outer(tpos, inv)                       # [T, 64]
    cosT = np.cos(fr).T.astype(np.float32)         # [64, T]
    sinT = np.sin(fr).T.astype(np.float32)
    cs2 = np.concatenate([cosT, cosT], axis=0).astype(NPBF16)     # [128, T]
    snpm = np.concatenate([-sinT, sinT], axis=0).astype(NPBF16)   # [128, T]

    tri = (np.arange(128)[None, :] >= np.arange(128)[:, None]).astype(NPBF16)
    iden = np.eye(128, dtype=np.float32)
    ones = np.ones((128, 128), dtype=np.float32).astype(NPBF16)

    scale = np.float32(1.0 / np.sqrt(HD))
    # xb[b]: [128, tb*8192 + e*512 + c] = x[b][tb*512+c, e*128+p]
    xbs = []
    for b in range(B):
        xT = x[b].T                                  # [E, T]
        v = xT.reshape(NE, 128, NTB, TB).transpose(1, 2, 0, 3)
        xbs.append(np.ascontiguousarray(v.reshape(128, NTB * NE * TB)).astype(NPBF16))

    in_maps = []
    for i in range(N_CORES):
        b, g = i // 4, i % 4
        rows = []
        for h in range(HPG):
            base = (g * HPG + h) * HD
            rows.append(Wq[base + _PERM, :])
        wq_c = (np.concatenate(rows, axis=0) * scale).T  # [E, 512]
        wk_c = Wk[g * HD + _PERM, :].T                   # [E, 128]
        wv_c = Wv[g * HD:(g + 1) * HD, :].T              # [E, 128]
        # wo blocks (hh, eo): [128, (hh*4+eo)*512 + c] = WoT[hh*128+p, eo*512+c]
        wo_c = Wo[:, g * 512:(g + 1) * 512].T            # [512, E]
        wo_p = np.ascontiguousarray(
            wo_c.reshape(HPG, 128, 4, 512).transpose(1, 0, 2, 3).reshape(128, NE * 512)
        ).astype(NPBF16)
        in_maps.append({
            "xb": xbs[b],
            "wq": _pack_w(wq_c),
            "wk": _pack_w(wk_c),
            "wv": _pack_w(wv_c),
            "wo": wo_p,
            "cs2": cs2, "snpm": snpm, "tri": tri, "iden": iden,
            "ones": ones,
        })
    return in_maps


_NC = None


def build_nc():
    global _NC
    if _NC is not None:
        return _NC
    nc = bacc.Bacc("TRN2", target_bir_lowering=False, debug=False,
                   num_devices=N_CORES)
    ins = {
        "xb": nc.dram_tensor("xb", [128, NTB * NE * TB], BF16, kind="ExternalInput").ap(),
        "wq": nc.dram_tensor("wq", [128, NE * 512], BF16, kind="ExternalInput").ap(),
        "wk": nc.dram_tensor("wk", [128, NE * 128], BF16, kind="ExternalInput").ap(),
        "wv": nc.dram_tensor("wv", [128, NE * 128], BF16, kind="ExternalInput").ap(),
        "wo": nc.dram_tensor("wo", [128, NE * 512], BF16, kind="ExternalInput").ap(),
        "cs2": nc.dram_tensor("cs2", [128, T], BF16, kind="ExternalInput").ap(),
        "snpm": nc.dram_tensor("snpm", [128, T], BF16, kind="ExternalInput").ap(),
        "tri": nc.dram_tensor("tri", [128, 128], BF16, kind="ExternalInput").ap(),
        "iden": nc.dram_tensor("iden", [128, 128], F32, kind="ExternalInput").ap(),
        "ones": nc.dram_tensor("ones", [128, 128], BF16, kind="ExternalInput").ap(),
    }
    out = nc.dram_tensor("out", [T, E], BF16, kind="ExternalOutput").ap()
    with tile.TileContext(nc) as tc:
        build_body(tc, out, ins)
    nc.compile()
    _NC = nc
    return nc


def gather(results):
    """results: list of 8 dicts with 'out' [T, E] bf16 partials -> [B, T, E] f32."""
    out = np.zeros((B, T, E), dtype=np.float32)
    for i in range(N_CORES):
        out[i // 4] += np.asarray(results[i]["out"], dtype=np.float32)
    return out


def kernel(**inputs):
    nc = build_nc()
    in_maps = host_prep(inputs)
    res = run_bass_kernel_spmd(nc, in_maps, core_ids=list(range(N_CORES)))
    return gather(res.results)


if __name__ == "__main__":
    rng = np.random.default_rng(0)
    ins = {
        "x": rng.standard_normal((B, T, E), dtype=np.float32),
        "Wq": rng.standard_normal((E, E), dtype=np.float32) * 0.02,
        "Wk": rng.standard_normal((NG * HD, E), dtype=np.float32) * 0.02,
        "Wv": rng.standard_normal((NG * HD, E), dtype=np.float32) * 0.02,
        "Wo": rng.standard_normal((E, E), dtype=np.float32) * 0.02,
    }
    out = kernel(**ins)
    print(out.shape, out.dtype, np.abs(out).mean())
